# revision 40
# baseline (speedup 1.0000x reference)
"""GlobalPointer RE-decoder kernel for 8 trn2 NeuronCores.

Problem: x = concat(hidden_states, emb_table[entity_labels]) [B=4, S=2048, 1024];
for 3 weight sets: proj = x @ W.T + b -> split q|k (64 each);
logits = (q @ k.T) * SCALE; out = logits * pad - (1-pad)*INF  (pad broadcast
over the query axis). Output [4, 3, 2048, 2048] f32 (~201 MB) -> memory bound.

Sharding: core c -> (batch b = c//2, query-half h = c%2). Each core computes
[3, 1024, 2048] of the output. The SPMD program is identical on all cores; the
query-half selection is achieved by swapping the token order of the inputs for
odd cores (queries are always tokens 0:1024 of the core's xt), and swapping the
key (column) axis of those cores' outputs back on the host.

Two device programs:
 - fast path (attention_mask all ones, the spec'd case): fp16 matmul inputs
   (PE at 1 cycle/row instead of fp32's 4; rel err ~6e-4 vs the 2e-2 gate),
   no mask work at all. x and W are converted to fp16 on the host (halves
   the input HBM reads); accumulation stays fp32 in PSUM. Scores are staged
   in SBUF as fp16 AND stored to DRAM as fp16 (the host upcasts to f32
   during the gather — values are identical since scores are fp16-rounded
   in SBUF either way), halving the 24 MB/core output write stream.
   PSUM->SBUF copies are split 2:1 across DVE and ACT. Input loads issue
   from ACT (HWDGE) so they interleave with the output stores (SP) instead
   of queueing behind them. All tile pools persist across in-NEFF reps with
   bufs>=2, so consecutive reps pipeline. Steady state is PE-bound at the
   matmul-row floor (~41 us/core in CoreSim; ~48-70 us/core measured on HW,
   tenancy-dependent; the all-f32 baseline was ~198 us sim / ~267 us HW).
 - general path (any mask): the original all-f32 kernel, which folds the pad
   mask into the score matmul via a 65th contraction row.
"""

import sys

if "/opt/trn_rl_repo" not in sys.path:
    sys.path.insert(0, "/opt/trn_rl_repo")

import numpy as np

HIDDEN = 992
LABEL_EMB = 32
TOTAL = 1024          # feature dim seen by the pointer heads
HEAD = 64             # head size (q and k each)
NW = 3                # head / tail / t2h
B = 4
S = 2048
SH = S // 2           # per-core query rows
INF = 1e12
SCALE = 1.0 / 8.0     # 1/sqrt(64), exact in fp32
KC = TOTAL // 128     # 8 contraction chunks for the projection
NJ = S // 512         # 4 free-dim chunks of 512
NG = 5                # packed projection groups (see _emit_fast)
WCOLS = 4 * 128 + HEAD  # 576 weight columns incl. duplicated k0/k1

_CACHE = {}


# ---------------------------------------------------------------------------
# fast path: mask-free, fp16 matmul inputs
# ---------------------------------------------------------------------------

def _emit_fast(nc, bass, pools, xh_d, wh_d, bqk_d, out_d, out16,
               in_eng="scalar", copy_split=0, out_hybrid=0, out_dram16=True):
    from concourse import mybir

    f32 = mybir.dt.float32
    f16 = mybir.dt.float16
    cpool, qkpool, xpool, ppool, spool, opool = pools
    in_dma = {"scalar": nc.scalar.dma_start,
              "sync": nc.sync.dma_start,
              "gpsimd": nc.gpsimd.dma_start}[in_eng]

    # inputs are DMA'd from the ACT engine so their data phases interleave
    # with the previous rep's output stores (issued from SP) instead of
    # queueing behind them
    wh_sb = [cpool.tile([128, WCOLS], f16, name=f"wh{k}", tag=f"wh{k}")
             for k in range(KC)]
    bqk_sb = cpool.tile([128, NG], f32, name="bqk", tag="bqk")
    xh_sb = [xpool.tile([128, S], f16, name=f"xh{k}", tag=f"xh{k}")
             for k in range(KC)]

    # qt: [65, S] fp16, rows 0:64 = q*SCALE + bq ; kt likewise = k + bk
    # (65 partitions force base partition 0 so score-matmul operands share
    # a base; row 64 is unused)
    qt = [qkpool.tile([HEAD + 1, S], f16, name=f"qt{w}", tag=f"qt{w}")
          for w in range(NW)]
    kt = [qkpool.tile([HEAD + 1, S], f16, name=f"kt{w}", tag=f"kt{w}")
          for w in range(NW)]

    # interleave xh/wh loads so the k-accumulation can start after the first
    # pair lands rather than after the whole input set
    for k in range(KC):
        in_dma(xh_sb[k][:], xh_d.ap()[k * 128:(k + 1) * 128, :])
        in_dma(wh_sb[k][:], wh_d.ap()[k * 128:(k + 1) * 128, :])
    in_dma(bqk_sb[:], bqk_d.ap())

    # Projection passes. Queries are only needed for this core's own query
    # half (tokens 0:SH) — the q rows for tokens SH:S would be dead work.
    # Head-pairs are packed into 128-wide output groups so each pass fills
    # the PE output dim; k0/k1 weight columns are duplicated on the host so
    # every group is contiguous in wh. 10 passes instead of 12.
    #   group -> (col offset, M, token-chunk js, [(dest tile, psum rows)])
    groups = [
        (0 * 128, 128, (0, 1), [(qt[0], 0), (qt[1], HEAD)]),
        (1 * 128, 128, (0, 1), [(qt[2], 0), (kt[0], HEAD)]),
        (2 * 128, 128, (0, 1), [(kt[1], 0), (kt[2], HEAD)]),
        (3 * 128, 128, (2, 3), [(kt[0], 0), (kt[1], HEAD)]),
        (4 * 128, HEAD, (2, 3), [(kt[2], 0)]),
    ]
    for gi, (col, m, jlist, dests) in enumerate(groups):
        for j in jlist:
            pp = ppool.tile([128, 512], f32, name="pp", tag="pp")
            for k in range(KC):
                nc.tensor.matmul(
                    pp[0:m, :],
                    wh_sb[k][:, col:col + m],
                    xh_sb[k][:, j * 512:(j + 1) * 512],
                    start=(k == 0),
                    stop=(k == KC - 1),
                )
            js = bass.ts(j, 512)
            # ACT bias-add, f32 PSUM -> fp16 SBUF
            for dest, row in dests:
                nc.scalar.add(dest[0:HEAD, js], pp[row:row + HEAD, :],
                              bqk_sb[row:row + HEAD, gi:gi + 1])

    # ---- scores: out[w, m, n] = q~[:, m] . k~[:, n]
    # out16: stage scores in SBUF as fp16 and upcast to f32 during the
    # store DMA (SWDGE cast path) — halves the SBUF-side read bytes of the
    # 24 MB output stream
    copy_idx = 0
    tile_idx = 0
    for w in range(NW):
        for mi in range(SH // 128):
            # out_dram16: DRAM output is fp16 (host upcasts during gather;
            # values are identical — scores are fp16-rounded in SBUF either
            # way) and the store is a plain HWDGE DMA. Halves HBM writes.
            # out_hybrid=N: every N-th output tile is staged f32 and stored
            # via HWDGE (SP) instead of the SWDGE fp16-cast path, engaging
            # both DGE paths concurrently
            f32_store = out_hybrid and tile_idx % out_hybrid == 0
            tile_idx += 1
            use16 = (out16 or out_dram16) and not f32_store
            osb = opool.tile([128, S], f16 if use16 else f32,
                             name="osb", tag="osbf" if f32_store else "osb")
            lhsT = qt[w][0:HEAD, mi * 128:(mi + 1) * 128]
            for nh in range(2):
                sp = spool.tile([128, 1024], f32, name="sp", tag="sp")
                for ns in range(2):
                    col = nh * 1024 + ns * 512
                    nc.tensor.matmul(
                        sp[:, ns * 512:(ns + 1) * 512],
                        lhsT,
                        kt[w][0:HEAD, col:col + 512],
                        start=True,
                        stop=True,
                    )
                # PSUM -> SBUF copies; copy_split=N sends every (N+1)-th
                # copy to ACT, the rest to DVE (0 = all DVE)
                oslice = osb[:, nh * 1024:(nh + 1) * 1024]
                if copy_split and copy_idx % (copy_split + 1) == copy_split:
                    nc.scalar.copy(oslice, sp[:])
                else:
                    nc.vector.tensor_copy(oslice, sp[:])
                copy_idx += 1
            dst = out_d.ap()[w, mi * 128:(mi + 1) * 128, :]
            if out_dram16:
                nc.sync.dma_start(dst, osb[:])      # fp16 -> fp16, HWDGE
            elif use16:
                nc.gpsimd.dma_start(dst, osb[:])    # fp16 -> f32 SWDGE cast
            else:
                nc.sync.dma_start(dst, osb[:])


def _build_fast(reps=1, out16=True, in_eng="gpsimd", copy_split=2,
                out_hybrid=0, out_dram16=True):
    import concourse.bass as bass
    import concourse.tile as tile
    from concourse import bacc, mybir

    f32 = mybir.dt.float32
    f16 = mybir.dt.float16
    nc = bacc.Bacc("TRN2", target_bir_lowering=False, debug=False)

    xh_d = nc.dram_tensor("xh", [TOTAL, S], f16, kind="ExternalInput")
    wh_d = nc.dram_tensor("wh", [TOTAL, WCOLS], f16, kind="ExternalInput")
    bqk_d = nc.dram_tensor("bqk", [128, NG], f32, kind="ExternalInput")
    out_d = nc.dram_tensor("out", [NW, SH, S],
                           f16 if out_dram16 else f32, kind="ExternalOutput")

    # pools live for the whole program: tiles cycle through bufs across reps,
    # so rep r+1's loads double-buffer against rep r's consumers instead of
    # serializing on SBUF reuse
    with tile.TileContext(nc) as tc:
        with (
            tc.tile_pool(name="const", bufs=2) as cpool,
            tc.tile_pool(name="qk", bufs=2) as qkpool,
            tc.tile_pool(name="xh", bufs=2) as xpool,
            tc.tile_pool(name="ppsum", bufs=2, space="PSUM") as ppool,
            tc.tile_pool(name="spsum", bufs=3, space="PSUM") as spool,
            tc.tile_pool(name="osb", bufs=4) as opool,
        ):
            pools = (cpool, qkpool, xpool, ppool, spool, opool)
            for rep in range(reps):
                _emit_fast(nc, bass, pools, xh_d, wh_d, bqk_d, out_d, out16,
                           in_eng=in_eng, copy_split=copy_split,
                           out_hybrid=out_hybrid, out_dram16=out_dram16)

    nc.compile()
    return nc


def _prep_inputs_fast(hidden_states, entity_labels, attention_mask, emb_table,
                      W_head, b_head, W_tail, b_tail, W_t2h, b_t2h):
    hs = np.asarray(hidden_states, dtype=np.float32)
    labels = np.asarray(entity_labels)
    emb = np.asarray(emb_table, dtype=np.float32)

    lab = emb[labels]                                   # [B,S,32]
    x = np.concatenate([hs, lab], axis=-1)              # [B,S,1024] f32

    Ws = [np.asarray(W, dtype=np.float32) for W in (W_head, W_tail, W_t2h)]
    bs = [np.asarray(b, dtype=np.float32) for b in (b_head, b_tail, b_t2h)]
    q = [Ws[w][:HEAD] * SCALE for w in range(NW)]       # [64, 1024] each
    kk = [Ws[w][HEAD:] for w in range(NW)]
    qb = [bs[w][:HEAD] * SCALE for w in range(NW)]
    kb = [bs[w][HEAD:] for w in range(NW)]
    # packed projection groups: [q0|q1], [q2|k0], [k1|k2], [k0|k1], [k2|0]
    # (matches the `groups` table in _emit_fast)
    Wcat = np.zeros((WCOLS, TOTAL), np.float32)
    bqk = np.zeros((128, NG), np.float32)
    packs = [(q[0], q[1], qb[0], qb[1]), (q[2], kk[0], qb[2], kb[0]),
             (kk[1], kk[2], kb[1], kb[2]), (kk[0], kk[1], kb[0], kb[1]),
             (kk[2], None, kb[2], None)]
    for g, (wa, wb, ba, bb) in enumerate(packs):
        Wcat[g * 128:g * 128 + HEAD] = wa
        bqk[0:HEAD, g] = ba
        if wb is not None:
            Wcat[g * 128 + HEAD:g * 128 + 128] = wb
            bqk[HEAD:128, g] = bb
    wh = np.ascontiguousarray(Wcat.T.astype(np.float16))  # [1024, 576] fp16

    in_maps = []
    for c in range(8):
        b, h = divmod(c, 2)
        xh = x[b].T.astype(np.float16)                  # [1024, 2048] fp16
        if h:
            xh = np.concatenate([xh[:, SH:], xh[:, :SH]], axis=1)
        in_maps.append({
            "xh": np.ascontiguousarray(xh),
            "wh": wh,
            "bqk": bqk,
        })
    return in_maps


# ---------------------------------------------------------------------------
# general path: original all-f32 kernel with the mask folded into the matmul
# ---------------------------------------------------------------------------

def _emit_once(nc, tc, bass, f32, rep, xt_d, wt_d, bq_d, bk_d, padb_d, crow_d, out_d):
    r = f"r{rep}_"
    with (
        tc.tile_pool(name=r + "const", bufs=1) as cpool,
        tc.tile_pool(name=r + "qk", bufs=1) as qkpool,
    ):
        wt_sb = [cpool.tile([128, NW * 128], f32, name=f"{r}wt{k}", tag=f"wt{k}")
                 for k in range(KC)]
        bq_sb = cpool.tile([HEAD, NW], f32, name=r + "bq", tag="bq")
        bk_sb = cpool.tile([HEAD, NW], f32, name=r + "bk", tag="bk")
        padb_sb = cpool.tile([HEAD, S], f32, name=r + "padb", tag="padb")
        for k in range(KC):
            nc.sync.dma_start(wt_sb[k][:], wt_d.ap()[k * 128:(k + 1) * 128, :])
        nc.sync.dma_start(bq_sb[:], bq_d.ap())
        nc.sync.dma_start(bk_sb[:], bk_d.ap())
        nc.sync.dma_start(padb_sb[:], padb_d.ap())

        # q~ [65, S]: rows 0:64 = q*SCALE + bq, row 64 = ones
        # k~ [65, S]: rows 0:64 = (k + bk) * pad, row 64 = (pad-1)*INF
        qt = [qkpool.tile([HEAD + 1, S], f32, name=f"{r}qt{w}", tag=f"qt{w}")
              for w in range(NW)]
        kt = [qkpool.tile([HEAD + 1, S], f32, name=f"{r}kt{w}", tag=f"kt{w}")
              for w in range(NW)]
        for w in range(NW):
            nc.gpsimd.memset(qt[w][HEAD:HEAD + 1, :], 1.0)
            nc.sync.dma_start(kt[w][HEAD:HEAD + 1, :], crow_d.ap())

        # ---- projection: projT[w] = W~[w] @ x.T, built per 512-col chunk
        with (
            tc.tile_pool(name=r + "xt", bufs=1) as xpool,
            tc.tile_pool(name=r + "ppsum", bufs=4, space="PSUM") as ppool,
            tc.tile_pool(name=r + "ktmp", bufs=4) as tpool,
        ):
            xt_sb = [xpool.tile([128, S], f32, name=f"{r}xt{k}", tag=f"xt{k}")
                     for k in range(KC)]
            for k in range(KC):
                nc.sync.dma_start(xt_sb[k][:], xt_d.ap()[k * 128:(k + 1) * 128, :])

            for w in range(NW):
                for j in range(NJ):
                    pp = ppool.tile([128, 512], f32, name=r + "pp", tag="pp")
                    for k in range(KC):
                        nc.tensor.matmul(
                            pp[:],
                            wt_sb[k][:, w * 128:(w + 1) * 128],
                            xt_sb[k][:, j * 512:(j + 1) * 512],
                            start=(k == 0),
                            stop=(k == KC - 1),
                        )
                    js = bass.ts(j, 512)
                    # q rows: ACT copy with per-partition bias
                    nc.scalar.add(qt[w][0:HEAD, js], pp[0:HEAD, :], bq_sb[:, w:w + 1])
                    # k rows: ACT bias-add, then DVE multiply by pad
                    tmp = tpool.tile([HEAD, 512], f32, name=r + "tmp", tag="tmp")
                    nc.scalar.add(tmp[:], pp[HEAD:128, :], bk_sb[:, w:w + 1])
                    nc.vector.tensor_mul(kt[w][0:HEAD, js], tmp[:], padb_sb[:, js])

        # ---- scores: out[w, m, n] = q~[:, m] . k~[:, n]
        with (
            tc.tile_pool(name=r + "spsum", bufs=2, space="PSUM") as spool,
            tc.tile_pool(name=r + "osb", bufs=4) as opool,
        ):
            for w in range(NW):
                for mi in range(SH // 128):
                    osb = opool.tile([128, S], f32, name=r + "osb", tag="osb")
                    lhsT = qt[w][:, mi * 128:(mi + 1) * 128]
                    for nh in range(2):
                        sp = spool.tile([128, 1024], f32, name=r + "sp", tag="sp")
                        for ns in range(2):
                            col = nh * 1024 + ns * 512
                            nc.tensor.matmul(
                                sp[:, ns * 512:(ns + 1) * 512],
                                lhsT,
                                kt[w][:, col:col + 512],
                                start=True,
                                stop=True,
                            )
                        oslice = osb[:, nh * 1024:(nh + 1) * 1024]
                        if nh == 0:
                            nc.scalar.copy(oslice, sp[:])
                        else:
                            nc.vector.tensor_copy(oslice, sp[:])
                    nc.sync.dma_start(
                        out_d.ap()[w, mi * 128:(mi + 1) * 128, :], osb[:]
                    )


def _build(reps=1):
    import concourse.bass as bass
    import concourse.tile as tile
    from concourse import bacc, mybir

    f32 = mybir.dt.float32
    nc = bacc.Bacc("TRN2", target_bir_lowering=False, debug=False)

    xt_d = nc.dram_tensor("xt", [TOTAL, S], f32, kind="ExternalInput")
    wt_d = nc.dram_tensor("wt", [TOTAL, NW * 128], f32, kind="ExternalInput")
    bq_d = nc.dram_tensor("bq", [HEAD, NW], f32, kind="ExternalInput")
    bk_d = nc.dram_tensor("bk", [HEAD, NW], f32, kind="ExternalInput")
    padb_d = nc.dram_tensor("padb", [HEAD, S], f32, kind="ExternalInput")
    crow_d = nc.dram_tensor("crow", [1, S], f32, kind="ExternalInput")
    out_d = nc.dram_tensor("out", [NW, SH, S], f32, kind="ExternalOutput")

    with tile.TileContext(nc) as tc:
        for rep in range(reps):
            _emit_once(nc, tc, bass, f32, rep,
                       xt_d, wt_d, bq_d, bk_d, padb_d, crow_d, out_d)

    nc.compile()
    return nc


def _prep_inputs(hidden_states, entity_labels, attention_mask, emb_table,
                 W_head, b_head, W_tail, b_tail, W_t2h, b_t2h):
    hs = np.asarray(hidden_states, dtype=np.float32)
    labels = np.asarray(entity_labels)
    mask = np.asarray(attention_mask, dtype=np.float32)
    emb = np.asarray(emb_table, dtype=np.float32)

    lab = emb[labels]                                   # [B,S,32]
    x = np.concatenate([hs, lab], axis=-1)              # [B,S,1024] f32

    Ws = [np.asarray(W, dtype=np.float32) for W in (W_head, W_tail, W_t2h)]
    bs = [np.asarray(b, dtype=np.float32) for b in (b_head, b_tail, b_t2h)]
    Wcat = np.empty((NW * 128, TOTAL), np.float32)
    bq = np.empty((HEAD, NW), np.float32)
    bk = np.empty((HEAD, NW), np.float32)
    for w in range(NW):
        Wcat[w * 128:w * 128 + HEAD] = Ws[w][:HEAD] * SCALE
        Wcat[w * 128 + HEAD:(w + 1) * 128] = Ws[w][HEAD:]
        bq[:, w] = bs[w][:HEAD] * SCALE
        bk[:, w] = bs[w][HEAD:]
    wt = np.ascontiguousarray(Wcat.T)                   # [1024, 384]

    in_maps = []
    for c in range(8):
        b, h = divmod(c, 2)
        xt = x[b].T                                     # [1024, 2048]
        m = mask[b]
        if h:
            xt = np.concatenate([xt[:, SH:], xt[:, :SH]], axis=1)
            m = np.concatenate([m[SH:], m[:SH]])
        in_maps.append({
            "xt": np.ascontiguousarray(xt),
            "wt": wt,
            "bq": bq,
            "bk": bk,
            "padb": np.ascontiguousarray(np.broadcast_to(m, (HEAD, S))),
            "crow": ((m - 1.0) * INF).reshape(1, S),
        })
    return in_maps


# ---------------------------------------------------------------------------
# host entry
# ---------------------------------------------------------------------------

def _gather(results):
    out = np.empty((B, NW, S, S), np.float32)
    for c in range(8):
        b, h = divmod(c, 2)
        o = results[c]["out"]                           # [3, 1024, 2048]
        if h:
            o = np.concatenate([o[..., SH:], o[..., :SH]], axis=-1)
        # fp16 device output upcasts on assignment into the f32 array
        out[b, :, h * SH:(h + 1) * SH, :] = o
    return out


def kernel(**inputs) -> np.ndarray:
    from concourse.bass_utils import run_bass_kernel_spmd

    mask = np.asarray(inputs["attention_mask"], dtype=np.float32)
    if np.all(mask == 1.0):
        if "nc_fast" not in _CACHE:
            _CACHE["nc_fast"] = _build_fast()
        nc = _CACHE["nc_fast"]
        in_maps = _prep_inputs_fast(**inputs)
    else:
        if "nc" not in _CACHE:
            _CACHE["nc"] = _build()
        nc = _CACHE["nc"]
        in_maps = _prep_inputs(**inputs)

    res = run_bass_kernel_spmd(nc, in_maps, list(range(8)))
    return _gather(res.results)


# revision 41
# speedup vs baseline: 1.0679x; 1.0679x over previous
"""GlobalPointer RE-decoder kernel for 8 trn2 NeuronCores.

Problem: x = concat(hidden_states, emb_table[entity_labels]) [B=4, S=2048, 1024];
for 3 weight sets: proj = x @ W.T + b -> split q|k (64 each);
logits = (q @ k.T) * SCALE; out = logits * pad - (1-pad)*INF  (pad broadcast
over the query axis). Output [4, 3, 2048, 2048] f32 (~201 MB) -> memory bound.

Sharding: core c -> (batch b = c//2, query-half h = c%2). Each core computes
[3, 1024, 2048] of the output. The SPMD program is identical on all cores; the
query-half selection is achieved by swapping the token order of the inputs for
odd cores (queries are always tokens 0:1024 of the core's xt), and swapping the
key (column) axis of those cores' outputs back on the host.

Two device programs:
 - fast path (attention_mask all ones, the spec'd case): fp16 matmul inputs
   (PE at 1 cycle/row instead of fp32's 4; rel err ~6e-4 vs the 2e-2 gate),
   no mask work at all. x and W are converted to fp16 on the host (halves
   the input HBM reads); accumulation stays fp32 in PSUM. Scores are staged
   in SBUF as fp16 AND stored to DRAM as fp16 (the host upcasts to f32
   during the gather — values are identical since scores are fp16-rounded
   in SBUF either way), halving the 24 MB/core output write stream.
   PSUM->SBUF copies are split 2:1 across DVE and ACT. Input loads issue
   from ACT (HWDGE) so they interleave with the output stores (SP) instead
   of queueing behind them. All tile pools persist across in-NEFF reps with
   bufs>=2, so consecutive reps pipeline. Steady state is PE-bound at the
   matmul-row floor (~41 us/core in CoreSim; ~48-70 us/core measured on HW,
   tenancy-dependent; the all-f32 baseline was ~198 us sim / ~267 us HW).
 - general path (any mask): the original all-f32 kernel, which folds the pad
   mask into the score matmul via a 65th contraction row.
"""

import sys

if "/opt/trn_rl_repo" not in sys.path:
    sys.path.insert(0, "/opt/trn_rl_repo")

import numpy as np

HIDDEN = 992
LABEL_EMB = 32
TOTAL = 1024          # feature dim seen by the pointer heads
HEAD = 64             # head size (q and k each)
NW = 3                # head / tail / t2h
B = 4
S = 2048
SH = S // 2           # per-core query rows
INF = 1e12
SCALE = 1.0 / 8.0     # 1/sqrt(64), exact in fp32
KC = TOTAL // 128     # 8 contraction chunks for the projection
NJ = S // 512         # 4 free-dim chunks of 512
NG = 5                # packed projection groups (see _emit_fast)
WCOLS = 4 * 128 + HEAD  # 576 weight columns incl. duplicated k0/k1

_CACHE = {}


# ---------------------------------------------------------------------------
# fast path: mask-free, fp16 matmul inputs
# ---------------------------------------------------------------------------

def _emit_fast(nc, bass, pools, xh_d, wh_d, bqk_d, out_d, out16,
               in_eng="scalar", copy_split=0, out_hybrid=0, out_dram16=True):
    from concourse import mybir

    f32 = mybir.dt.float32
    f16 = mybir.dt.float16
    cpool, qkpool, xpool, ppool, spool, opool = pools
    in_dma = {"scalar": nc.scalar.dma_start,
              "sync": nc.sync.dma_start,
              "gpsimd": nc.gpsimd.dma_start}[in_eng]

    # inputs are DMA'd from the ACT engine so their data phases interleave
    # with the previous rep's output stores (issued from SP) instead of
    # queueing behind them
    wh_sb = [cpool.tile([128, WCOLS], f16, name=f"wh{k}", tag=f"wh{k}")
             for k in range(KC)]
    bqk_sb = cpool.tile([128, NG], f32, name="bqk", tag="bqk")
    xh_sb = [xpool.tile([128, S], f16, name=f"xh{k}", tag=f"xh{k}")
             for k in range(KC)]

    # qt: [65, S] fp16, rows 0:64 = q*SCALE + bq ; kt likewise = k + bk
    # (65 partitions force base partition 0 so score-matmul operands share
    # a base; row 64 is unused)
    qt = [qkpool.tile([HEAD + 1, S], f16, name=f"qt{w}", tag=f"qt{w}")
          for w in range(NW)]
    kt = [qkpool.tile([HEAD + 1, S], f16, name=f"kt{w}", tag=f"kt{w}")
          for w in range(NW)]

    # interleave xh/wh loads so the k-accumulation can start after the first
    # pair lands rather than after the whole input set
    for k in range(KC):
        in_dma(xh_sb[k][:], xh_d.ap()[k * 128:(k + 1) * 128, :])
        in_dma(wh_sb[k][:], wh_d.ap()[k * 128:(k + 1) * 128, :])
    in_dma(bqk_sb[:], bqk_d.ap())

    # Projection passes. Queries are only needed for this core's own query
    # half (tokens 0:SH) — the q rows for tokens SH:S would be dead work.
    # Head-pairs are packed into 128-wide output groups so each pass fills
    # the PE output dim; k0/k1 weight columns are duplicated on the host so
    # every group is contiguous in wh. 10 passes instead of 12.
    #   group -> (col offset, M, token-chunk js, [(dest tile, psum rows)])
    groups = [
        (0 * 128, 128, (0, 1), [(qt[0], 0), (qt[1], HEAD)]),
        (1 * 128, 128, (0, 1), [(qt[2], 0), (kt[0], HEAD)]),
        (2 * 128, 128, (0, 1), [(kt[1], 0), (kt[2], HEAD)]),
        (3 * 128, 128, (2, 3), [(kt[0], 0), (kt[1], HEAD)]),
        (4 * 128, HEAD, (2, 3), [(kt[2], 0)]),
    ]
    for gi, (col, m, jlist, dests) in enumerate(groups):
        for j in jlist:
            pp = ppool.tile([128, 512], f32, name="pp", tag="pp")
            for k in range(KC):
                nc.tensor.matmul(
                    pp[0:m, :],
                    wh_sb[k][:, col:col + m],
                    xh_sb[k][:, j * 512:(j + 1) * 512],
                    start=(k == 0),
                    stop=(k == KC - 1),
                )
            js = bass.ts(j, 512)
            # ACT bias-add, f32 PSUM -> fp16 SBUF
            for dest, row in dests:
                nc.scalar.add(dest[0:HEAD, js], pp[row:row + HEAD, :],
                              bqk_sb[row:row + HEAD, gi:gi + 1])

    # ---- scores: out[w, m, n] = q~[:, m] . k~[:, n]
    # out16: stage scores in SBUF as fp16 and upcast to f32 during the
    # store DMA (SWDGE cast path) — halves the SBUF-side read bytes of the
    # 24 MB output stream
    copy_idx = 0
    tile_idx = 0
    for w in range(NW):
        for mi in range(SH // 128):
            # out_dram16: DRAM output is fp16 (host upcasts during gather;
            # values are identical — scores are fp16-rounded in SBUF either
            # way) and the store is a plain HWDGE DMA. Halves HBM writes.
            # out_hybrid=N: every N-th output tile is staged f32 and stored
            # via HWDGE (SP) instead of the SWDGE fp16-cast path, engaging
            # both DGE paths concurrently
            f32_store = out_hybrid and tile_idx % out_hybrid == 0
            tile_idx += 1
            use16 = (out16 or out_dram16) and not f32_store
            osb = opool.tile([128, S], f16 if use16 else f32,
                             name="osb", tag="osbf" if f32_store else "osb")
            lhsT = qt[w][0:HEAD, mi * 128:(mi + 1) * 128]
            for nh in range(2):
                sp = spool.tile([128, 1024], f32, name="sp", tag="sp")
                for ns in range(2):
                    col = nh * 1024 + ns * 512
                    nc.tensor.matmul(
                        sp[:, ns * 512:(ns + 1) * 512],
                        lhsT,
                        kt[w][0:HEAD, col:col + 512],
                        start=True,
                        stop=True,
                    )
                # PSUM -> SBUF copies; copy_split=N sends every (N+1)-th
                # copy to ACT, the rest to DVE (0 = all DVE)
                oslice = osb[:, nh * 1024:(nh + 1) * 1024]
                if copy_split and copy_idx % (copy_split + 1) == copy_split:
                    nc.scalar.copy(oslice, sp[:])
                else:
                    nc.vector.tensor_copy(oslice, sp[:])
                copy_idx += 1
            dst = out_d.ap()[w, mi * 128:(mi + 1) * 128, :]
            if out_dram16:
                nc.sync.dma_start(dst, osb[:])      # fp16 -> fp16, HWDGE
            elif use16:
                nc.gpsimd.dma_start(dst, osb[:])    # fp16 -> f32 SWDGE cast
            else:
                nc.sync.dma_start(dst, osb[:])


def _build_fast(reps=1, out16=True, in_eng="scalar", copy_split=2,
                out_hybrid=0, out_dram16=True):
    import concourse.bass as bass
    import concourse.tile as tile
    from concourse import bacc, mybir

    f32 = mybir.dt.float32
    f16 = mybir.dt.float16
    nc = bacc.Bacc("TRN2", target_bir_lowering=False, debug=False)

    xh_d = nc.dram_tensor("xh", [TOTAL, S], f16, kind="ExternalInput")
    wh_d = nc.dram_tensor("wh", [TOTAL, WCOLS], f16, kind="ExternalInput")
    bqk_d = nc.dram_tensor("bqk", [128, NG], f32, kind="ExternalInput")
    out_d = nc.dram_tensor("out", [NW, SH, S],
                           f16 if out_dram16 else f32, kind="ExternalOutput")

    # pools live for the whole program: tiles cycle through bufs across reps,
    # so rep r+1's loads double-buffer against rep r's consumers instead of
    # serializing on SBUF reuse
    with tile.TileContext(nc) as tc:
        with (
            tc.tile_pool(name="const", bufs=2) as cpool,
            tc.tile_pool(name="qk", bufs=2) as qkpool,
            tc.tile_pool(name="xh", bufs=2) as xpool,
            tc.tile_pool(name="ppsum", bufs=2, space="PSUM") as ppool,
            tc.tile_pool(name="spsum", bufs=3, space="PSUM") as spool,
            tc.tile_pool(name="osb", bufs=4) as opool,
        ):
            pools = (cpool, qkpool, xpool, ppool, spool, opool)
            for rep in range(reps):
                _emit_fast(nc, bass, pools, xh_d, wh_d, bqk_d, out_d, out16,
                           in_eng=in_eng, copy_split=copy_split,
                           out_hybrid=out_hybrid, out_dram16=out_dram16)

    nc.compile()
    return nc


def _prep_inputs_fast(hidden_states, entity_labels, attention_mask, emb_table,
                      W_head, b_head, W_tail, b_tail, W_t2h, b_t2h):
    hs = np.asarray(hidden_states, dtype=np.float32)
    labels = np.asarray(entity_labels)
    emb = np.asarray(emb_table, dtype=np.float32)

    lab = emb[labels]                                   # [B,S,32]
    x = np.concatenate([hs, lab], axis=-1)              # [B,S,1024] f32

    Ws = [np.asarray(W, dtype=np.float32) for W in (W_head, W_tail, W_t2h)]
    bs = [np.asarray(b, dtype=np.float32) for b in (b_head, b_tail, b_t2h)]
    q = [Ws[w][:HEAD] * SCALE for w in range(NW)]       # [64, 1024] each
    kk = [Ws[w][HEAD:] for w in range(NW)]
    qb = [bs[w][:HEAD] * SCALE for w in range(NW)]
    kb = [bs[w][HEAD:] for w in range(NW)]
    # packed projection groups: [q0|q1], [q2|k0], [k1|k2], [k0|k1], [k2|0]
    # (matches the `groups` table in _emit_fast)
    Wcat = np.zeros((WCOLS, TOTAL), np.float32)
    bqk = np.zeros((128, NG), np.float32)
    packs = [(q[0], q[1], qb[0], qb[1]), (q[2], kk[0], qb[2], kb[0]),
             (kk[1], kk[2], kb[1], kb[2]), (kk[0], kk[1], kb[0], kb[1]),
             (kk[2], None, kb[2], None)]
    for g, (wa, wb, ba, bb) in enumerate(packs):
        Wcat[g * 128:g * 128 + HEAD] = wa
        bqk[0:HEAD, g] = ba
        if wb is not None:
            Wcat[g * 128 + HEAD:g * 128 + 128] = wb
            bqk[HEAD:128, g] = bb
    wh = np.ascontiguousarray(Wcat.T.astype(np.float16))  # [1024, 576] fp16

    in_maps = []
    for c in range(8):
        b, h = divmod(c, 2)
        xh = x[b].T.astype(np.float16)                  # [1024, 2048] fp16
        if h:
            xh = np.concatenate([xh[:, SH:], xh[:, :SH]], axis=1)
        in_maps.append({
            "xh": np.ascontiguousarray(xh),
            "wh": wh,
            "bqk": bqk,
        })
    return in_maps


# ---------------------------------------------------------------------------
# general path: original all-f32 kernel with the mask folded into the matmul
# ---------------------------------------------------------------------------

def _emit_once(nc, tc, bass, f32, rep, xt_d, wt_d, bq_d, bk_d, padb_d, crow_d, out_d):
    r = f"r{rep}_"
    with (
        tc.tile_pool(name=r + "const", bufs=1) as cpool,
        tc.tile_pool(name=r + "qk", bufs=1) as qkpool,
    ):
        wt_sb = [cpool.tile([128, NW * 128], f32, name=f"{r}wt{k}", tag=f"wt{k}")
                 for k in range(KC)]
        bq_sb = cpool.tile([HEAD, NW], f32, name=r + "bq", tag="bq")
        bk_sb = cpool.tile([HEAD, NW], f32, name=r + "bk", tag="bk")
        padb_sb = cpool.tile([HEAD, S], f32, name=r + "padb", tag="padb")
        for k in range(KC):
            nc.sync.dma_start(wt_sb[k][:], wt_d.ap()[k * 128:(k + 1) * 128, :])
        nc.sync.dma_start(bq_sb[:], bq_d.ap())
        nc.sync.dma_start(bk_sb[:], bk_d.ap())
        nc.sync.dma_start(padb_sb[:], padb_d.ap())

        # q~ [65, S]: rows 0:64 = q*SCALE + bq, row 64 = ones
        # k~ [65, S]: rows 0:64 = (k + bk) * pad, row 64 = (pad-1)*INF
        qt = [qkpool.tile([HEAD + 1, S], f32, name=f"{r}qt{w}", tag=f"qt{w}")
              for w in range(NW)]
        kt = [qkpool.tile([HEAD + 1, S], f32, name=f"{r}kt{w}", tag=f"kt{w}")
              for w in range(NW)]
        for w in range(NW):
            nc.gpsimd.memset(qt[w][HEAD:HEAD + 1, :], 1.0)
            nc.sync.dma_start(kt[w][HEAD:HEAD + 1, :], crow_d.ap())

        # ---- projection: projT[w] = W~[w] @ x.T, built per 512-col chunk
        with (
            tc.tile_pool(name=r + "xt", bufs=1) as xpool,
            tc.tile_pool(name=r + "ppsum", bufs=4, space="PSUM") as ppool,
            tc.tile_pool(name=r + "ktmp", bufs=4) as tpool,
        ):
            xt_sb = [xpool.tile([128, S], f32, name=f"{r}xt{k}", tag=f"xt{k}")
                     for k in range(KC)]
            for k in range(KC):
                nc.sync.dma_start(xt_sb[k][:], xt_d.ap()[k * 128:(k + 1) * 128, :])

            for w in range(NW):
                for j in range(NJ):
                    pp = ppool.tile([128, 512], f32, name=r + "pp", tag="pp")
                    for k in range(KC):
                        nc.tensor.matmul(
                            pp[:],
                            wt_sb[k][:, w * 128:(w + 1) * 128],
                            xt_sb[k][:, j * 512:(j + 1) * 512],
                            start=(k == 0),
                            stop=(k == KC - 1),
                        )
                    js = bass.ts(j, 512)
                    # q rows: ACT copy with per-partition bias
                    nc.scalar.add(qt[w][0:HEAD, js], pp[0:HEAD, :], bq_sb[:, w:w + 1])
                    # k rows: ACT bias-add, then DVE multiply by pad
                    tmp = tpool.tile([HEAD, 512], f32, name=r + "tmp", tag="tmp")
                    nc.scalar.add(tmp[:], pp[HEAD:128, :], bk_sb[:, w:w + 1])
                    nc.vector.tensor_mul(kt[w][0:HEAD, js], tmp[:], padb_sb[:, js])

        # ---- scores: out[w, m, n] = q~[:, m] . k~[:, n]
        with (
            tc.tile_pool(name=r + "spsum", bufs=2, space="PSUM") as spool,
            tc.tile_pool(name=r + "osb", bufs=4) as opool,
        ):
            for w in range(NW):
                for mi in range(SH // 128):
                    osb = opool.tile([128, S], f32, name=r + "osb", tag="osb")
                    lhsT = qt[w][:, mi * 128:(mi + 1) * 128]
                    for nh in range(2):
                        sp = spool.tile([128, 1024], f32, name=r + "sp", tag="sp")
                        for ns in range(2):
                            col = nh * 1024 + ns * 512
                            nc.tensor.matmul(
                                sp[:, ns * 512:(ns + 1) * 512],
                                lhsT,
                                kt[w][:, col:col + 512],
                                start=True,
                                stop=True,
                            )
                        oslice = osb[:, nh * 1024:(nh + 1) * 1024]
                        if nh == 0:
                            nc.scalar.copy(oslice, sp[:])
                        else:
                            nc.vector.tensor_copy(oslice, sp[:])
                    nc.sync.dma_start(
                        out_d.ap()[w, mi * 128:(mi + 1) * 128, :], osb[:]
                    )


def _build(reps=1):
    import concourse.bass as bass
    import concourse.tile as tile
    from concourse import bacc, mybir

    f32 = mybir.dt.float32
    nc = bacc.Bacc("TRN2", target_bir_lowering=False, debug=False)

    xt_d = nc.dram_tensor("xt", [TOTAL, S], f32, kind="ExternalInput")
    wt_d = nc.dram_tensor("wt", [TOTAL, NW * 128], f32, kind="ExternalInput")
    bq_d = nc.dram_tensor("bq", [HEAD, NW], f32, kind="ExternalInput")
    bk_d = nc.dram_tensor("bk", [HEAD, NW], f32, kind="ExternalInput")
    padb_d = nc.dram_tensor("padb", [HEAD, S], f32, kind="ExternalInput")
    crow_d = nc.dram_tensor("crow", [1, S], f32, kind="ExternalInput")
    out_d = nc.dram_tensor("out", [NW, SH, S], f32, kind="ExternalOutput")

    with tile.TileContext(nc) as tc:
        for rep in range(reps):
            _emit_once(nc, tc, bass, f32, rep,
                       xt_d, wt_d, bq_d, bk_d, padb_d, crow_d, out_d)

    nc.compile()
    return nc


def _prep_inputs(hidden_states, entity_labels, attention_mask, emb_table,
                 W_head, b_head, W_tail, b_tail, W_t2h, b_t2h):
    hs = np.asarray(hidden_states, dtype=np.float32)
    labels = np.asarray(entity_labels)
    mask = np.asarray(attention_mask, dtype=np.float32)
    emb = np.asarray(emb_table, dtype=np.float32)

    lab = emb[labels]                                   # [B,S,32]
    x = np.concatenate([hs, lab], axis=-1)              # [B,S,1024] f32

    Ws = [np.asarray(W, dtype=np.float32) for W in (W_head, W_tail, W_t2h)]
    bs = [np.asarray(b, dtype=np.float32) for b in (b_head, b_tail, b_t2h)]
    Wcat = np.empty((NW * 128, TOTAL), np.float32)
    bq = np.empty((HEAD, NW), np.float32)
    bk = np.empty((HEAD, NW), np.float32)
    for w in range(NW):
        Wcat[w * 128:w * 128 + HEAD] = Ws[w][:HEAD] * SCALE
        Wcat[w * 128 + HEAD:(w + 1) * 128] = Ws[w][HEAD:]
        bq[:, w] = bs[w][:HEAD] * SCALE
        bk[:, w] = bs[w][HEAD:]
    wt = np.ascontiguousarray(Wcat.T)                   # [1024, 384]

    in_maps = []
    for c in range(8):
        b, h = divmod(c, 2)
        xt = x[b].T                                     # [1024, 2048]
        m = mask[b]
        if h:
            xt = np.concatenate([xt[:, SH:], xt[:, :SH]], axis=1)
            m = np.concatenate([m[SH:], m[:SH]])
        in_maps.append({
            "xt": np.ascontiguousarray(xt),
            "wt": wt,
            "bq": bq,
            "bk": bk,
            "padb": np.ascontiguousarray(np.broadcast_to(m, (HEAD, S))),
            "crow": ((m - 1.0) * INF).reshape(1, S),
        })
    return in_maps


# ---------------------------------------------------------------------------
# host entry
# ---------------------------------------------------------------------------

def _gather(results):
    out = np.empty((B, NW, S, S), np.float32)
    for c in range(8):
        b, h = divmod(c, 2)
        o = results[c]["out"]                           # [3, 1024, 2048]
        if h:
            o = np.concatenate([o[..., SH:], o[..., :SH]], axis=-1)
        # fp16 device output upcasts on assignment into the f32 array
        out[b, :, h * SH:(h + 1) * SH, :] = o
    return out


def kernel(**inputs) -> np.ndarray:
    from concourse.bass_utils import run_bass_kernel_spmd

    mask = np.asarray(inputs["attention_mask"], dtype=np.float32)
    if np.all(mask == 1.0):
        if "nc_fast" not in _CACHE:
            _CACHE["nc_fast"] = _build_fast()
        nc = _CACHE["nc_fast"]
        in_maps = _prep_inputs_fast(**inputs)
    else:
        if "nc" not in _CACHE:
            _CACHE["nc"] = _build()
        nc = _CACHE["nc"]
        in_maps = _prep_inputs(**inputs)

    res = run_bass_kernel_spmd(nc, in_maps, list(range(8)))
    return _gather(res.results)
